# revision 40
# baseline (speedup 1.0000x reference)
"""Trainium2 Bass kernel for the AttentionBlock problem.

Problem (hardcoded): x (16, 512, 32, 32) fp32; GroupNorm(32 groups) ->
1x1-conv QKV (1536x512) -> 4-head attention over 1024 tokens, head dim 128
-> 1x1-conv proj (512x512) -> residual add.

Sharding: data-parallel over batch, 2 batches per core on 8 cores; params
replicated. Weights are pre-transposed on the host so every matmul operand
is consumed in its natural [contract-dim-on-partitions, free] layout.

Per-core dataflow (per batch):
  - GroupNorm: per-channel bn_stats/bn_aggr, group (16-channel) reduction
    and broadcast-back via tiny matmuls against constant group-membership
    masks (host inputs).
  - QKV: q, k produced as [d, n] (heads = 128-row chunks); v produced
    directly transposed as [n, d] by swapping the matmul operands.
  - Attention per head: T = K^T Q (keys on partitions), P^T = exp(T/sqrt(hd))
    on the scalar engine (no max subtraction needed: |T/sqrt(hd)| < ~7),
    O^T accumulated over key chunks. Softmax denominators via matmuls with
    an all-ones [128,128] stationary matrix, which lands the sums in PSUM
    replicated across partitions: reciprocal + multiply finish softmax with
    no broadcast step.
  - Proj + residual, streamed back to DRAM.
  - The two batches' phases are interleaved (batch-1 norm+qkv emitted
    before batch-0 proj) to hide the attention tail.

All matmuls run as float32r (fp32 storage, full-rate PE streaming; the BIR
verifier requires all matmul operands to be produced as float32r).
"""

import math

import numpy as np

import concourse.mybir as mybir
import concourse.tile as tile
from concourse import bacc
from concourse.bass_utils import run_bass_kernel_spmd

# Problem constants
B, C, N = 16, 512, 1024          # batch, channels, tokens (32*32)
HEADS, HD = 4, 128               # heads, head dim
GROUPS, GS = 32, 16              # norm groups, channels per group
EPS = 1e-5
N_CORES = 8
BL = B // N_CORES                # batches per core
CC = C // 128                    # channel chunks of 128
SCALE = 1.0 / math.sqrt(HD)

F32 = mybir.dt.float32
F32R = mybir.dt.float32r


def _mm(nc, out, lhsT, rhs, start=True, stop=True):
    nc.tensor.matmul(out, lhsT, rhs, start=start, stop=stop)


def build(reps=1):
    nc = bacc.Bacc("TRN2", target_bir_lowering=False, debug=False)

    x_d = nc.dram_tensor("x", [BL, C, N], F32, kind="ExternalInput").ap()
    nw_d = nc.dram_tensor("norm_w", [C], F32, kind="ExternalInput").ap()
    nb_d = nc.dram_tensor("norm_b", [C], F32, kind="ExternalInput").ap()
    wq_d = nc.dram_tensor("qkv_wT", [C, 3 * C], F32R,
                          kind="ExternalInput").ap()
    qb_d = nc.dram_tensor("qkv_b", [3 * C], F32, kind="ExternalInput").ap()
    wp_d = nc.dram_tensor("proj_wT", [C, C], F32R, kind="ExternalInput").ap()
    pb_d = nc.dram_tensor("proj_b", [C], F32, kind="ExternalInput").ap()
    gm_d = nc.dram_tensor("gmask", [CC, 128, GROUPS], F32R,
                          kind="ExternalInput").ap()
    gmT_d = nc.dram_tensor("gmaskT", [CC, GROUPS, 128], F32R,
                           kind="ExternalInput").ap()
    ones_d = nc.dram_tensor("ones", [128, 128], F32R,
                            kind="ExternalInput").ap()
    out_d = nc.dram_tensor("out", [BL, C, N], F32, kind="ExternalOutput").ap()

    with tile.TileContext(nc) as tc:
        with (
            nc.allow_low_precision(reason="fp32r tiles feeding fp32r matmuls"),
            tc.tile_pool(name="const", bufs=1) as const,
            tc.tile_pool(name="xp", bufs=2) as xp,
            tc.tile_pool(name="xnp", bufs=1) as xnp,
            tc.tile_pool(name="qkp", bufs=1) as qkp,
            tc.tile_pool(name="vtp", bufs=1) as vtp,
            tc.tile_pool(name="ptp", bufs=4) as ptp,
            tc.tile_pool(name="ocp", bufs=1) as ocp,
            tc.tile_pool(name="smallp", bufs=4) as smallp,
            tc.tile_pool(name="rbp", bufs=2) as rbp,
            tc.tile_pool(name="yp", bufs=4) as yp,
            tc.tile_pool(name="ps_work", bufs=4, space="PSUM") as ps_work,
            tc.tile_pool(name="ps_o", bufs=2, space="PSUM") as ps_o,
            tc.tile_pool(name="ps_sum", bufs=2, space="PSUM") as ps_sum,
        ):
            # ---- constants / weights (loaded once) ----
            # small consts + masks first: they gate the first stats matmuls
            w_sb = const.tile([128, CC], F32, name="w_sb")
            nc.scalar.dma_start(
                out=w_sb, in_=nw_d.rearrange("(cc p) -> p cc", p=128))
            b_sb = const.tile([128, CC], F32, name="b_sb")
            nc.scalar.dma_start(
                out=b_sb, in_=nb_d.rearrange("(cc p) -> p cc", p=128))
            qb_sb = const.tile([128, 8], F32, name="qb_sb")
            nc.scalar.dma_start(
                out=qb_sb, in_=qb_d[0:2 * C].rearrange("(oc p) -> p oc", p=128)
            )
            pb_sb = const.tile([128, CC], F32, name="pb_sb")
            nc.scalar.dma_start(
                out=pb_sb, in_=pb_d.rearrange("(cc p) -> p cc", p=128))

            ones_mat = const.tile([128, 128], F32R, name="ones_mat")
            nc.scalar.dma_start(out=ones_mat, in_=ones_d)
            eps_t = const.tile([GROUPS, 1], F32, name="eps_t")
            nc.vector.memset(eps_t, EPS)

            gm = []
            gmT = []
            for cc in range(CC):
                t = const.tile([128, GROUPS], F32R, name=f"gm{cc}")
                nc.scalar.dma_start(out=t, in_=gm_d[cc])
                tT = const.tile([GROUPS, 128], F32R, name=f"gmT{cc}")
                nc.scalar.dma_start(out=tT, in_=gmT_d[cc])
                gm.append(t)
                gmT.append(tT)

            # v bias broadcast across all token partitions: [128, 512]
            vb_bc = const.tile([128, C], F32, name="vb_bc")
            nc.scalar.dma_start(
                out=vb_bc, in_=qb_d[2 * C:3 * C].partition_broadcast(128)
            )

            wq_sb = []
            for cc in range(CC):
                t = const.tile([128, 3 * C], F32R, name=f"wq{cc}")
                nc.sync.dma_start(out=t, in_=wq_d[cc * 128:(cc + 1) * 128, :])
                wq_sb.append(t)
            wp_sb = []
            for cc in range(CC):
                t = const.tile([128, C], F32R, name=f"wp{cc}")
                nc.scalar.dma_start(
                    out=t, in_=wp_d[cc * 128:(cc + 1) * 128, :])
                wp_sb.append(t)

            # ---- per batch pipeline (phases interleaved across batches) ----
            def norm_qkv(b):
                x_t = xp.tile([128, CC, N], F32, tag="x", name=f"x_t{b}")
                xr = x_d[b].rearrange("(cc p) n -> p cc n", p=128)
                for cc in range(CC):
                    nc.gpsimd.dma_start(out=x_t[:, cc, :], in_=xr[:, cc, :])

                # group stats via per-channel bn_stats/bn_aggr
                cols = smallp.tile([128, CC, 2], F32R, tag="mv",
                                   name=f"cols{b}")
                for cc in range(CC):
                    stats = smallp.tile([128, 2, 6], F32, tag="stats",
                                        name=f"stats{b}_{cc}")
                    for s in range(2):
                        nc.vector.bn_stats(
                            out=stats[:, s, :],
                            in_=x_t[:, cc, s * 512:(s + 1) * 512],
                        )
                    mv_f = smallp.tile([128, 2], F32, tag="mvf",
                                       name=f"mvf{b}_{cc}")
                    nc.vector.bn_aggr(out=mv_f, in_=stats)
                    # mv[:,1] := E[x^2] = var + mean^2
                    msq = smallp.tile([128, 1], F32, tag="msq",
                                      name=f"msq{b}_{cc}")
                    nc.vector.tensor_mul(msq, mv_f[:, 0:1], mv_f[:, 0:1])
                    nc.vector.tensor_add(mv_f[:, 1:2], mv_f[:, 1:2], msq)
                    nc.vector.tensor_copy(out=cols[:, cc, :], in_=mv_f)

                gstats = ps_sum.tile([GROUPS, 2], F32, tag="s",
                                     name=f"gstats{b}")
                for cc in range(CC):
                    _mm(nc, gstats, gm[cc], cols[:, cc, :],
                        start=(cc == 0), stop=(cc == CC - 1))
                grp = smallp.tile([GROUPS, 2], F32R, tag="grp", name=f"grp{b}")
                nc.scalar.mul(out=grp, in_=gstats, mul=1.0 / GS)
                gvar = smallp.tile([GROUPS, 1], F32, tag="gvar",
                                   name=f"gvar{b}")
                nc.vector.tensor_mul(gvar, grp[:, 0:1], grp[:, 0:1])
                nc.vector.tensor_sub(gvar, grp[:, 1:2], gvar)
                nc.scalar.activation(
                    out=gvar, in_=gvar,
                    func=mybir.ActivationFunctionType.Sqrt,
                    bias=eps_t, scale=1.0,
                )
                nc.vector.reciprocal(out=grp[:, 1:2], in_=gvar)

                # broadcast per-group (mean, rstd) back to channels, normalize
                xn_t = xnp.tile([128, CC, N], F32R, tag="xn", name=f"xn{b}")
                for cc in range(CC):
                    bc = ps_sum.tile([128, 2], F32, tag="s",
                                     name=f"bc{b}_{cc}")
                    _mm(nc, bc, gmT[cc], grp)
                    a_t = smallp.tile([128, 1], F32, tag="a_t",
                                      name=f"a_t{b}_{cc}")
                    nc.vector.tensor_mul(a_t, bc[:, 1:2], w_sb[:, cc:cc + 1])
                    b_t = smallp.tile([128, 1], F32, tag="b_t",
                                      name=f"b_t{b}_{cc}")
                    nc.vector.tensor_mul(b_t, bc[:, 0:1], a_t)
                    nc.vector.tensor_sub(b_t, b_sb[:, cc:cc + 1], b_t)
                    nc.vector.tensor_scalar(
                        out=xn_t[:, cc, :], in0=x_t[:, cc, :],
                        scalar1=a_t, scalar2=b_t,
                        op0=mybir.AluOpType.mult, op1=mybir.AluOpType.add,
                    )

                # qkv
                q_t = qkp.tile([128, HEADS, N], F32R, tag="q", name=f"q{b}")
                k_t = qkp.tile([128, HEADS, N], F32R, tag="k", name=f"k{b}")
                for oc in range(8):  # q: 0..3, k: 4..7
                    dst = q_t if oc < 4 else k_t
                    h = oc % 4
                    accs = [ps_work.tile([128, 512], F32, tag="w",
                                         name=f"qkacc{b}_{oc}_{s}")
                            for s in range(2)]
                    for cc in range(CC):
                        for s in range(2):  # consecutive mms share lhsT
                            _mm(nc, accs[s],
                                wq_sb[cc][:, oc * 128:(oc + 1) * 128],
                                xn_t[:, cc, s * 512:(s + 1) * 512],
                                start=(cc == 0), stop=(cc == CC - 1))
                    for s in range(2):
                        nc.vector.tensor_scalar(
                            out=dst[:, h, s * 512:(s + 1) * 512], in0=accs[s],
                            scalar1=qb_sb[:, oc:oc + 1], scalar2=None,
                            op0=mybir.AluOpType.add,
                        )
                vt_t = vtp.tile([128, 8, C], F32R, tag="vt", name=f"vt{b}")
                for tc_i in range(8):
                    acc = ps_work.tile([128, 512], F32, tag="w",
                                       name=f"vacc{b}_{tc_i}")
                    for cc in range(CC):
                        _mm(nc, acc,
                            xn_t[:, cc, tc_i * 128:(tc_i + 1) * 128],
                            wq_sb[cc][:, 2 * C:3 * C],
                            start=(cc == 0), stop=(cc == CC - 1))
                    nc.vector.tensor_add(vt_t[:, tc_i, :], acc, vb_bc)
                return x_t, q_t, k_t, vt_t

            def attn(b, q_t, k_t, vt_t):
                ocat = ocp.tile([128, HEADS, N], F32R, tag="ocat",
                                name=f"ocat{b}")
                state = {}

                def head_state(h):
                    if h not in state:
                        state[h] = {
                            "o": [ps_o.tile([128, 512], F32, tag="o",
                                            name=f"o{b}_{h}_{i}")
                                  for i in range(2)],
                            "s": [ps_sum.tile([128, 512], F32, tag="s",
                                              name=f"s{b}_{h}_{i}")
                                  for i in range(2)],
                            "pt": [],
                        }
                    return state[h]

                def emit_t(h, mc):
                    st = head_state(h)
                    pt = ptp.tile([128, N], F32R, tag="pt",
                                  name=f"pt{b}_{h}_{mc}")
                    st["pt"].append(pt)
                    for s in range(2):
                        tps = ps_work.tile([128, 512], F32, tag="w",
                                           name=f"t{b}_{h}_{mc}_{s}")
                        _mm(nc, tps,
                            k_t[:, h, mc * 128:(mc + 1) * 128],
                            q_t[:, h, s * 512:(s + 1) * 512])
                        nc.scalar.activation(
                            out=pt[:, s * 512:(s + 1) * 512], in_=tps,
                            func=mybir.ActivationFunctionType.Exp,
                            scale=SCALE,
                        )

                def emit_av(h, mc):
                    st = head_state(h)
                    pt = st["pt"][mc]
                    for s in range(2):
                        _mm(nc, st["o"][s],
                            vt_t[:, mc, h * HD:(h + 1) * HD],
                            pt[:, s * 512:(s + 1) * 512],
                            start=(mc == 0), stop=(mc == 7))
                    for s in range(2):
                        _mm(nc, st["s"][s], ones_mat,
                            pt[:, s * 512:(s + 1) * 512],
                            start=(mc == 0), stop=(mc == 7))

                def emit_tail(h):
                    st = state[h]
                    # sums are replicated across partitions: reciprocal and
                    # multiply straight out of PSUM, no broadcast needed
                    rb_sb = rbp.tile([128, N], F32, tag="rb",
                                     name=f"rb{b}_{h}")
                    for s in range(2):
                        nc.vector.reciprocal(
                            out=rb_sb[:, s * 512:(s + 1) * 512],
                            in_=st["s"][s],
                        )
                        nc.vector.tensor_mul(
                            ocat[:, h, s * 512:(s + 1) * 512], st["o"][s],
                            rb_sb[:, s * 512:(s + 1) * 512],
                        )

                # flat software pipeline across all (h, mc) tasks: AV lags T
                # by 3 slots, crossing head boundaries so each head's tail
                # hides behind the next head's first T matmuls
                tasks = [(h, mc) for h in range(HEADS) for mc in range(8)]
                LAG = 3
                for i, (h, mc) in enumerate(tasks):
                    emit_t(h, mc)
                    if i >= LAG:
                        ph, pmc = tasks[i - LAG]
                        emit_av(ph, pmc)
                        if pmc == 7:
                            emit_tail(ph)
                for i in range(len(tasks) - LAG, len(tasks)):
                    ph, pmc = tasks[i]
                    emit_av(ph, pmc)
                    if pmc == 7:
                        emit_tail(ph)
                return ocat

            def proj(b, x_t, ocat):
                for oc in range(CC):
                    accs = [ps_work.tile([128, 512], F32, tag="w",
                                         name=f"pacc{b}_{oc}_{s}")
                            for s in range(2)]
                    for cc in range(CC):
                        for s in range(2):  # consecutive mms share lhsT
                            _mm(nc, accs[s],
                                wp_sb[cc][:, oc * 128:(oc + 1) * 128],
                                ocat[:, cc, s * 512:(s + 1) * 512],
                                start=(cc == 0), stop=(cc == CC - 1))
                    for s in range(2):
                        ty = yp.tile([128, 512], F32, tag="ty",
                                     name=f"ty{b}_{oc}_{s}")
                        nc.scalar.activation(
                            out=ty, in_=accs[s],
                            func=mybir.ActivationFunctionType.Identity,
                            bias=pb_sb[:, oc:oc + 1], scale=1.0,
                        )
                        y = yp.tile([128, 512], F32, tag="y",
                                    name=f"y{b}_{oc}_{s}")
                        nc.vector.tensor_add(
                            y, ty, x_t[:, oc, s * 512:(s + 1) * 512]
                        )
                        nc.gpsimd.dma_start(
                            out=out_d[b, oc * 128:(oc + 1) * 128,
                                      s * 512:(s + 1) * 512],
                            in_=y,
                        )

            def body():
                st0 = norm_qkv(0)
                oc0 = attn(0, st0[1], st0[2], st0[3])
                st1 = norm_qkv(1)
                proj(0, st0[0], oc0)
                oc1 = attn(1, st1[1], st1[2], st1[3])
                proj(1, st1[0], oc1)

            if reps == 1:
                body()
            else:
                with tc.For_i(0, reps, 1):
                    body()

    nc.compile()
    return nc


_CACHE = {}


def _get_nc():
    if "nc" not in _CACHE:
        _CACHE["nc"] = build()
    return _CACHE["nc"]


def _gmasks():
    gm = np.zeros((CC, 128, GROUPS), np.float32)
    for cc in range(CC):
        for p in range(128):
            gm[cc, p, (cc * 128 + p) // GS] = 1.0
    gmT = np.ascontiguousarray(gm.transpose(0, 2, 1))
    return gm, gmT


def kernel(x, norm_w, norm_b, qkv_w, qkv_b, proj_w, proj_b):
    nc = _get_nc()
    x = np.asarray(x, dtype=np.float32).reshape(B, C, N)
    norm_w = np.ascontiguousarray(np.asarray(norm_w, dtype=np.float32))
    norm_b = np.ascontiguousarray(np.asarray(norm_b, dtype=np.float32))
    qkv_wT = np.ascontiguousarray(np.asarray(qkv_w, dtype=np.float32).T)
    qkv_b = np.ascontiguousarray(np.asarray(qkv_b, dtype=np.float32))
    proj_wT = np.ascontiguousarray(np.asarray(proj_w, dtype=np.float32).T)
    proj_b = np.ascontiguousarray(np.asarray(proj_b, dtype=np.float32))

    gm_np, gmT_np = _gmasks()
    ones_np = np.ones((128, 128), np.float32)
    in_maps = []
    for c in range(N_CORES):
        in_maps.append({
            "x": np.ascontiguousarray(x[c * BL:(c + 1) * BL]),
            "norm_w": norm_w,
            "norm_b": norm_b,
            "qkv_wT": qkv_wT,
            "qkv_b": qkv_b,
            "proj_wT": proj_wT,
            "proj_b": proj_b,
            "gmask": gm_np,
            "gmaskT": gmT_np,
            "ones": ones_np,
        })
    res = run_bass_kernel_spmd(nc, in_maps, core_ids=list(range(N_CORES)))
    out = np.concatenate([res.results[c]["out"] for c in range(N_CORES)],
                         axis=0)
    return out.reshape(B, C, 32, 32).astype(np.float32)
